# revision 6
# baseline (speedup 1.0000x reference)
"""Llama attention layer (B=1, S=2048, H=32, KVH=8, D=128, HID=4096) on 8 TRN2
NeuronCores.

Sharding: tensor-parallel over head groups. Core c computes Q heads
[4c..4c+4) and KV head c end-to-end (QKV projection, RoPE, causal GQA
attention, o_proj rows for its heads); chunked ReduceScatters sum the o_proj
partials. The host reassembles the full [2048, 4096] output.

v4 design (evolved from v3 after trace analysis):
  - Two HW DMA rings: wqkv weight groups + consts + wo on the Scalar
    engine's HWDGE queue (no-wait descriptors, enqueued at t~0); the hT
    stream / RoPE swaps / partial+out writes on the Sync queue.
  - Host pre-packs wqkv/hT so every DMA is contiguous (2-3 KB lines).
  - Attention inner loop runs the QK matmul one tile ahead of the
    exp -> PV chain (ps_sc bufs=3) so PV never waits on ACT latency.
  - All attention-phase PSUM evictions run on DVE; ACT does only exp.
  - o_proj filler paced over 75% of each chunk and stp deepened so the
    partial-write backlog during an in-flight ReduceScatter never blocks
    the DVE evictions (v3 lost ~9 us to that once per chunk).
  - Chunks 0-2 use one whole-chunk RS each (fewer ncfw dispatch floors);
    the last chunk is processed as two 256-row sub-blocks so its first
    o_proj half + RS fire while the PE is still busy with the second.
  - 2 MB dummy RS, gated on the tail of the weight load, warms ncfw at
    the real transfer size during the DMA-quiet part of phase A.
"""

import sys

if "/opt/trn_rl_repo" not in sys.path:
    sys.path.insert(0, "/opt/trn_rl_repo")

import numpy as np

# Model dims (hardcoded per problem spec)
H, KVH, D, HID = 32, 8, 128, 4096
S = 2048
THETA = 10000.0
NCORES = 8
QH = H // NCORES          # 4 query heads per core
P = 128                   # partitions
SC = 512                  # sequence chunk (matmul free dim)
NS = S // SC              # 4 chunks
KT = HID // P             # 32 contraction tiles for the projections
ST = S // P               # 16 sequence tiles of 128
NQK = QH + 2              # col-tiles per core in wqkv: q0..q3, k, v
WCOLS = NQK * P           # 768
GK = 2                    # weight k-tiles per DMA group
NG = KT // GK             # 16 groups
ISQRT_D = float(D) ** -0.5

_CACHE = {}


def _build():
    import concourse.bass as bass
    import concourse.tile as tile
    from concourse import bacc, mybir
    from contextlib import ExitStack

    F32 = mybir.dt.float32
    F16 = mybir.dt.float16
    AF = mybir.ActivationFunctionType

    nc = bacc.Bacc(
        "TRN2",
        target_bir_lowering=False,
        debug=False,
        enable_asserts=False,
        num_devices=NCORES,
    )

    # hT packed: [p, (c*KT + k)*SC + j] = h[c*SC + j, k*P + p]
    hT = nc.dram_tensor("hT", [P, NS * KT * SC], F16, kind="ExternalInput").ap()
    # wqkv packed: [p, k*WCOLS + n] = wq_c[k*P + p, n]
    wqkv = nc.dram_tensor("wqkv", [P, KT * WCOLS], F16, kind="ExternalInput").ap()
    wo = nc.dram_tensor("wo", [QH * D, HID], F16, kind="ExternalInput").ap()
    cos2 = nc.dram_tensor("cos2", [P, S], F16, kind="ExternalInput").ap()
    sinn2 = nc.dram_tensor("sinn2", [P, S], F16, kind="ExternalInput").ap()
    maskd = nc.dram_tensor("maskd", [P, 4 * P], F16, kind="ExternalInput").ap()
    ident = nc.dram_tensor("ident", [P, P], F16, kind="ExternalInput").ap()
    onesd = nc.dram_tensor("onesd", [P, 1], F16, kind="ExternalInput").ap()
    out = nc.dram_tensor("out", [S // NCORES, HID], F16, kind="ExternalOutput").ap()
    # partials: whole-chunk for chunks 0..2, two halves for the last chunk.
    # Separate DRAM tensors so whole-tensor WAR tracking never serializes
    # later o_proj DMA writes behind an in-flight ReduceScatter.
    partW = [nc.dram_tensor(f"part{c}", [SC, HID], F16).ap() for c in range(NS - 1)]
    rsW = [nc.dram_tensor(f"rs{c}", [64, HID], F16).ap() for c in range(NS - 1)]
    partH3 = [
        nc.dram_tensor(f"part3{hh}", [SC // 2, HID], F16).ap() for hh in range(2)
    ]
    rsH3 = [nc.dram_tensor(f"rs3{hh}", [32, HID], F16).ap() for hh in range(2)]
    # dummy collective: warms ncfw at the real transfer size; gated on the
    # weight-load tail so it moves data only once the DMA rings are quiet.
    dum_in = nc.dram_tensor("dum_in", [SC, HID // 2], F16).ap()
    dum_out = nc.dram_tensor("dum_out", [64, HID // 2], F16).ap()

    groups = [list(range(NCORES))]
    half = P // 2

    with tile.TileContext(nc) as tc:
        with ExitStack() as ctx:
            # ---------------- SBUF pools (whole-kernel lifetime) -------------
            cpool = ctx.enter_context(tc.tile_pool(name="const", bufs=1))
            ppool = ctx.enter_context(tc.tile_pool(name="persist", bufs=1))
            wpool = ctx.enter_context(tc.tile_pool(name="wres", bufs=1))
            h_pool = ctx.enter_context(tc.tile_pool(name="ht", bufs=8))
            rpool = ctx.enter_context(tc.tile_pool(name="ropet", bufs=3))
            ep = ctx.enter_context(tc.tile_pool(name="expp", bufs=6))
            esp = ctx.enter_context(tc.tile_pool(name="esump", bufs=2))
            sp = ctx.enter_context(tc.tile_pool(name="smallp", bufs=2))
            stp = ctx.enter_context(tc.tile_pool(name="stagep", bufs=32))

            mask_t = cpool.tile([P, 4 * P], F16, name="mask_t")
            ident_t = cpool.tile([P, P], F16, name="ident_t")
            ones_t = cpool.tile([P, 1], F16, name="ones_t")
            cos_t = cpool.tile([P, S], F16, name="cos_t")
            sin_t = cpool.tile([P, S], F16, name="sin_t")

            qk = {}
            for n in range(QH + 1):
                for c in range(NS):
                    qk[(n, c)] = ppool.tile(
                        [P, SC], F16, name=f"qk{n}_{c}", tag=f"qk{n}_{c}"
                    )
            vT = [
                ppool.tile([P, SC], F16, name=f"vT{c}", tag=f"vT{c}")
                for c in range(NS)
            ]
            vnat = [
                ppool.tile([P, P], F16, name=f"vn{t}", tag=f"vn{t}")
                for t in range(ST)
            ]
            attnT = {}
            for h in range(QH):
                for c in range(NS):
                    attnT[(h, c)] = ppool.tile(
                        [P, SC], F16, name=f"at{h}_{c}", tag=f"at{h}_{c}"
                    )

            wo_t = [
                wpool.tile([P, HID], F16, name=f"wo{hh}", tag=f"wo{hh}")
                for hh in range(QH)
            ]
            wq_g = {}

            def wslice(k, n):
                g, kk = divmod(k, GK)
                off = kk * WCOLS + n * P
                return wq_g[g][:, off : off + P]

            def emit_rope(c):
                csl = cos_t[:, c * SC : (c + 1) * SC]
                ssl = sin_t[:, c * SC : (c + 1) * SC]
                for n in range(QH + 1):
                    src = qk[(n, c)]
                    swp = rpool.tile([P, SC], F16, name="swp", tag="swp")
                    t1 = rpool.tile([P, SC], F16, name="t1", tag="t1")
                    nc.sync.dma_start(swp[0:half, :], src[half:P, :])
                    nc.sync.dma_start(swp[half:P, :], src[0:half, :])
                    nc.vector.tensor_mul(t1[:], src[:], csl)
                    nc.vector.tensor_mul(swp[:], swp[:], ssl)
                    nc.vector.tensor_add(src[:], t1[:], swp[:])

            # ---------------- stage A: QKV projection + RoPE + vT ------------
            with tc.tile_pool(name="psA", bufs=NQK, space="PSUM") as psA, tc.tile_pool(
                name="pst", bufs=2, space="PSUM"
            ) as pst:
                for c in range(NS):
                    # hT stream for this chunk on the Sync HWDGE queue
                    ht_tiles = []
                    for kk in range(KT // 2):
                        t = h_pool.tile([P, 2 * SC], F16, name="ht_t", tag="ht")
                        off = (c * KT + 2 * kk) * SC
                        nc.sync.dma_start(t[:], hT[:, off : off + 2 * SC])
                        ht_tiles.append(t)
                    if c == 0:
                        # weight stream + consts + wo on the Scalar HWDGE
                        # queue: none of these wait on anything, so ACT rips
                        # through them at t~0 and the rings drain together.
                        for g in range(NG):
                            wq_g[g] = wpool.tile(
                                [P, GK * WCOLS], F16, name=f"wqg{g}", tag=f"wqg{g}"
                            )
                            nc.scalar.dma_start(
                                wq_g[g][:],
                                wqkv[:, g * GK * WCOLS : (g + 1) * GK * WCOLS],
                            )
                        nc.scalar.dma_start(mask_t[:], maskd)
                        nc.scalar.dma_start(ident_t[:], ident)
                        nc.scalar.dma_start(ones_t[:], onesd)
                        nc.scalar.dma_start(cos_t[:], cos2)
                        nc.scalar.dma_start(sin_t[:], sinn2)
                        for hh in range(QH):
                            nc.scalar.dma_start(
                                wo_t[hh][:], wo[hh * P : (hh + 1) * P, :]
                            )
                        # gate the warmup collective on the weight-load tail
                        nc.sync.dma_start(
                            dum_in[0:1, 0:P], wq_g[NG - 1][0:1, 0:P]
                        )
                        nc.gpsimd.collective_compute(
                            "ReduceScatter",
                            mybir.AluOpType.add,
                            replica_groups=groups,
                            ins=[dum_in[:, :]],
                            outs=[dum_out[:, :]],
                        )
                    ps = [
                        psA.tile([P, SC], F32, name=f"psA{n}", tag="psA")
                        for n in range(NQK)
                    ]
                    for k in range(KT):
                        src = ht_tiles[k // 2][:, (k % 2) * SC : (k % 2 + 1) * SC]
                        for n in range(NQK):
                            nc.tensor.matmul(
                                ps[n][:],
                                wslice(k, n),
                                src,
                                start=(k == 0),
                                stop=(k == KT - 1),
                            )
                    # v first so the transposes can start earliest
                    nc.scalar.copy(vT[c][:], ps[NQK - 1][:])
                    for n in range(QH + 1):
                        nc.scalar.copy(qk[(n, c)][:], ps[n][:])
                    # RoPE for the PREVIOUS chunk: its swap DMAs sit in the
                    # Sync ring after this chunk's hT tiles.
                    if c > 0:
                        emit_rope(c - 1)

                # v transpose to natural [s, d]
                for t in range(ST):
                    c, j = divmod(t, NS)
                    tp = pst.tile([P, P], F16, name="tp", tag="tp")
                    nc.tensor.transpose(
                        tp[:], vT[c][:, j * P : (j + 1) * P], ident_t[:]
                    )
                    nc.vector.tensor_copy(vnat[t][:], tp[:])
                emit_rope(NS - 1)

            # ---------- attention + o_proj + reduce-scatter ----------
            with tc.tile_pool(name="pssc", bufs=3, space="PSUM") as ps_sc, tc.tile_pool(
                name="pssm", bufs=1, space="PSUM"
            ) as ps_sm, tc.tile_pool(
                name="pspv", bufs=2, space="PSUM"
            ) as ps_pv, tc.tile_pool(
                name="psop", bufs=2, space="PSUM"
            ) as ps_op:

                def emit_oproj_group(c, jj, nn):
                    op = ps_op.tile([P, SC], F32, name="op", tag="op")
                    for h in range(QH):
                        nc.tensor.matmul(
                            op[:],
                            attnT[(h, c)][:, jj * P : (jj + 1) * P],
                            wo_t[h][:, nn * SC : (nn + 1) * SC],
                            start=(h == 0),
                            stop=(h == QH - 1),
                        )
                    st = stp.tile([P, SC], F16, name="st", tag="st")
                    nc.vector.tensor_copy(st[:], op[:])
                    if c < NS - 1:
                        dst, row = partW[c], jj * P
                    else:
                        dst, row = partH3[jj // 2], (jj % 2) * P
                    nc.sync.dma_start(
                        dst[row : row + P, nn * SC : (nn + 1) * SC], st[:]
                    )

                def oproj_items(c, jjs):
                    return [
                        (lambda cc=c, j=jj, n=nn: emit_oproj_group(cc, j, n))
                        for jj in jjs
                        for nn in range(HID // SC)
                    ]

                def emit_rs(ins_ap, outs_ap):
                    nc.gpsimd.collective_compute(
                        "ReduceScatter",
                        mybir.AluOpType.add,
                        replica_groups=groups,
                        ins=[ins_ap],
                        outs=[outs_ap],
                    )

                def attention_block(c, q0, W, nsk, diag0, filler, rs_at, end_frac):
                    """Process q-columns [q0:q0+W) of chunk c (all QH heads).

                    filler: list of closures (o_proj groups) paced over the
                    first end_frac of the block's steps. rs_at: {count: fn}
                    fired when that many filler items have been emitted.
                    """
                    total_steps = QH * nsk
                    oi = 0
                    si = 0
                    pending = {}

                    def qk_mm(h, t):
                        kc, kj = divmod(t, NS)
                        ktile = qk[(QH, kc)][:, kj * P : (kj + 1) * P]
                        d = t - diag0
                        lo = d * P if d >= 0 else 0
                        sc_ps = ps_sc.tile([P, SC], F32, name="sc_ps", tag="sc")
                        nc.tensor.matmul(
                            sc_ps[:, lo:W], ktile, qk[(h, c)][:, q0 + lo : q0 + W],
                            start=True, stop=True,
                        )
                        pending[t] = (sc_ps, lo)

                    denom_pending = [None]

                    def emit_denom(h, esum, pv):
                        sm = ps_sm.tile([1, SC], F32, name="sm", tag="sm")
                        nc.tensor.matmul(
                            sm[:, 0:W], ones_t[:], esum[:, 0:W],
                            start=True, stop=True,
                        )
                        smh = sp.tile([1, SC], F32, name="smh", tag="smh")
                        rcp = sp.tile([1, SC], F32, name="rcp", tag="rcp")
                        bc = sp.tile([P, SC], F32, name="bc", tag="bc")
                        nc.scalar.copy(smh[:, 0:W], sm[:, 0:W])
                        nc.vector.reciprocal_approx_fast(rcp[:, 0:W], smh[:, 0:W])
                        nc.gpsimd.partition_broadcast(bc[:, 0:W], rcp[:, 0:W])
                        nc.vector.tensor_mul(
                            attnT[(h, c)][:, q0 : q0 + W], pv[:, 0:W], bc[:, 0:W]
                        )

                    for h in range(QH):
                        esum = esp.tile([P, SC], F16, name="esum", tag="esum")
                        pv = ps_pv.tile([P, SC], F32, name="pv", tag="pv")
                        qk_mm(h, 0)
                        # previous head's softmax denominator: emitted after
                        # this head's first QK so the PE never stalls on the
                        # esum chain at a head boundary.
                        if denom_pending[0] is not None:
                            denom_pending[0]()
                            denom_pending[0] = None
                        for t in range(nsk):
                            if t + 1 < nsk:
                                qk_mm(h, t + 1)
                            sc_ps, lo = pending.pop(t)
                            e = ep.tile([P, SC], F16, name="e", tag="e")
                            nc.scalar.activation(
                                e[:, lo:W], sc_ps[:, lo:W], AF.Exp, scale=ISQRT_D
                            )
                            if t >= diag0:
                                nc.vector.tensor_mul(
                                    e[:, lo : lo + P],
                                    e[:, lo : lo + P],
                                    mask_t[:, 3 * P : 4 * P],
                                )
                            if t == 0:
                                nc.vector.tensor_copy(esum[:, 0:W], e[:, 0:W])
                            else:
                                nc.vector.tensor_add(
                                    esum[:, lo:W], esum[:, lo:W], e[:, lo:W]
                                )
                            nc.tensor.matmul(
                                pv[:, lo:W], vnat[t][:], e[:, lo:W],
                                start=(t == 0), stop=(t == nsk - 1),
                            )
                            si += 1
                            start_si = 6
                            end_si = max(
                                start_si + 1, int(total_steps * end_frac)
                            )
                            frac = (si - start_si) / (end_si - start_si)
                            want = int(len(filler) * min(max(frac, 0.0), 1.0))
                            while oi < want:
                                filler[oi]()
                                oi += 1
                                if oi in rs_at:
                                    rs_at[oi]()
                        denom_pending[0] = (
                            lambda hh=h, es=esum, pp=pv: emit_denom(hh, es, pp)
                        )
                    denom_pending[0]()
                    while oi < len(filler):
                        filler[oi]()
                        oi += 1
                        if oi in rs_at:
                            rs_at[oi]()

                # chunks 0..2: full 512-wide blocks; chunk c's filler is the
                # previous chunk's o_proj, whole-chunk RS when complete.
                for c in range(NS - 1):
                    filler = (
                        oproj_items(c - 1, range(QH)) if c > 0 else []
                    )
                    rs_at = (
                        {32: (lambda cc=c - 1: emit_rs(
                            partW[cc][:, :], rsW[cc][:, :]))}
                        if c > 0 else {}
                    )
                    attention_block(
                        c, 0, SC, QH * c + QH, QH * c, filler, rs_at, 0.75
                    )

                # last chunk: two 256-wide sub-blocks. Sub 0 carries chunk 2's
                # o_proj; sub 1 carries the last chunk's OWN first o_proj half
                # so its RS fires while the PE is still busy.
                attention_block(
                    NS - 1, 0, 256, ST - 2, ST - 4,
                    oproj_items(NS - 2, range(QH)),
                    {32: lambda: emit_rs(partW[NS - 2][:, :], rsW[NS - 2][:, :])},
                    0.75,
                )
                attention_block(
                    NS - 1, 256, 256, ST, ST - 2,
                    oproj_items(NS - 1, (0, 1)),
                    {16: lambda: emit_rs(partH3[0][:, :], rsH3[0][:, :])},
                    0.5,
                )
                for fn in oproj_items(NS - 1, (2, 3)):
                    fn()
                emit_rs(partH3[1][:, :], rsH3[1][:, :])

            # deferred output copies at the tail of the Sync ring
            for c in range(NS - 1):
                nc.sync.dma_start(out[c * 64 : (c + 1) * 64, :], rsW[c][:, :])
            for hh in range(2):
                nc.sync.dma_start(
                    out[192 + hh * 32 : 224 + hh * 32, :], rsH3[hh][:, :]
                )

    nc.compile()
    return nc


def _get_nc():
    if "nc" not in _CACHE:
        _CACHE["nc"] = _build()
    return _CACHE["nc"]


def _host_inputs(positions, hidden_states, Wqkv, Wo):
    """Shard + relayout the full inputs for the 8 cores (fp16 device side)."""
    pos = np.asarray(positions).reshape(-1).astype(np.float64)  # [S]
    hs = np.asarray(hidden_states, dtype=np.float32).reshape(S, HID)
    Wqkv = np.asarray(Wqkv, dtype=np.float32)
    Wo = np.asarray(Wo, dtype=np.float32)

    # hT packed: [p, (c*KT + k)*SC + j] = hs[c*SC + j, k*P + p]
    hTp = np.ascontiguousarray(
        hs.reshape(NS, SC, KT, P).transpose(3, 0, 2, 1).reshape(P, NS * KT * SC)
    ).astype(np.float16)

    half = D // 2
    inv_freq = 1.0 / (THETA ** (np.arange(half, dtype=np.float64) / half))
    ang = pos[None, :] * inv_freq[:, None]  # [64, S]
    cos = np.cos(ang)
    sin = np.sin(ang)
    cos2 = np.ascontiguousarray(np.concatenate([cos, cos], axis=0)).astype(
        np.float16
    )
    sinn2 = np.ascontiguousarray(np.concatenate([-sin, sin], axis=0)).astype(
        np.float16
    )

    # causal mask, [sk, sq] orientation: [zeros(128x384) | upper-tri(128x128)].
    maskd = np.concatenate(
        [np.zeros((P, 3 * P), dtype=np.float16),
         np.triu(np.ones((P, P), dtype=np.float16))], axis=1)
    ident = np.eye(P, dtype=np.float16)
    onesd = np.ones((P, 1), dtype=np.float16)

    qb = Wqkv[:, : H * D]
    kb = Wqkv[:, H * D : H * D + KVH * D]
    vb = Wqkv[:, H * D + KVH * D :]

    in_maps = []
    for c in range(NCORES):
        wq_c = np.concatenate(
            [
                qb[:, c * QH * D : (c + 1) * QH * D],
                kb[:, c * D : (c + 1) * D],
                vb[:, c * D : (c + 1) * D],
            ],
            axis=1,
        )
        # packed: [p, k*WCOLS + n] = wq_c[k*P + p, n]
        wq_p = np.ascontiguousarray(
            wq_c.reshape(KT, P, WCOLS).transpose(1, 0, 2).reshape(P, KT * WCOLS)
        ).astype(np.float16)
        wo_c = Wo[c * QH * D : (c + 1) * QH * D, :].astype(np.float16)
        in_maps.append(
            {
                "hT": hTp,
                "wqkv": wq_p,
                "wo": np.ascontiguousarray(wo_c),
                "cos2": cos2,
                "sinn2": sinn2,
                "maskd": maskd,
                "ident": ident,
                "onesd": onesd,
            }
        )
    return in_maps


def _assemble(results):
    full = np.empty((S, HID), dtype=np.float32)
    for r in range(NCORES):
        oc = np.asarray(results[r]["out"], dtype=np.float32)  # [256, HID]
        for c in range(NS - 1):
            full[SC * c + 64 * r : SC * c + 64 * (r + 1), :] = oc[
                64 * c : 64 * (c + 1), :
            ]
        for hh in range(2):
            base = SC * (NS - 1) + 256 * hh + 32 * r
            full[base : base + 32, :] = oc[192 + hh * 32 : 224 + hh * 32, :]
    return full.reshape(1, S, HID)


def kernel(positions, hidden_states, Wqkv, Wo):
    from concourse.bass_utils import run_bass_kernel_spmd

    nc = _get_nc()
    in_maps = _host_inputs(positions, hidden_states, Wqkv, Wo)
    res = run_bass_kernel_spmd(nc, in_maps, core_ids=list(range(NCORES)))
    return _assemble(res.results)


def kernel_timed(positions, hidden_states, Wqkv, Wo, tmpdir="/tmp/ntff_trace"):
    """Like kernel() but with NTFF profiling; returns (output, exec_time_ns)."""
    import os
    import shutil

    from concourse.bass_utils import run_bass_kernel_spmd

    shutil.rmtree(tmpdir, ignore_errors=True)
    os.makedirs(tmpdir, exist_ok=True)
    nc = _get_nc()
    in_maps = _host_inputs(positions, hidden_states, Wqkv, Wo)
    res = run_bass_kernel_spmd(
        nc, in_maps, core_ids=list(range(NCORES)), trace=True, tmpdir=tmpdir
    )
    return _assemble(res.results), res.exec_time_ns


# revision 12
# speedup vs baseline: 1.0070x; 1.0070x over previous
"""Llama attention layer (B=1, S=2048, H=32, KVH=8, D=128, HID=4096) on 8 TRN2
NeuronCores.

Sharding: tensor-parallel over head groups. Core c computes Q heads
[4c..4c+4) and KV head c end-to-end (QKV projection, RoPE, causal GQA
attention, o_proj rows for its heads); chunked ReduceScatters sum the o_proj
partials. The host reassembles the full [2048, 4096] output.

v4 design (evolved from v3 after trace analysis):
  - Two HW DMA rings: wqkv weight groups + consts + wo on the Scalar
    engine's HWDGE queue (no-wait descriptors, enqueued at t~0); the hT
    stream / RoPE swaps / partial+out writes on the Sync queue.
  - Host pre-packs wqkv/hT so every DMA is contiguous (2-3 KB lines).
  - Attention inner loop runs the QK matmul one tile ahead of the
    exp -> PV chain (ps_sc bufs=3) so PV never waits on ACT latency.
  - All attention-phase PSUM evictions run on DVE; ACT does only exp.
  - o_proj filler paced over 75% of each chunk and stp deepened so the
    partial-write backlog during an in-flight ReduceScatter never blocks
    the DVE evictions (v3 lost ~9 us to that once per chunk).
  - Chunks 0-2 use one whole-chunk RS each (fewer ncfw dispatch floors);
    the last chunk is processed as two 256-row sub-blocks so its first
    o_proj half + RS fire while the PE is still busy with the second.
  - 2 MB dummy RS, gated on the tail of the weight load, warms ncfw at
    the real transfer size during the DMA-quiet part of phase A.
"""

import sys

if "/opt/trn_rl_repo" not in sys.path:
    sys.path.insert(0, "/opt/trn_rl_repo")

import numpy as np

# Model dims (hardcoded per problem spec)
H, KVH, D, HID = 32, 8, 128, 4096
S = 2048
THETA = 10000.0
NCORES = 8
QH = H // NCORES          # 4 query heads per core
P = 128                   # partitions
SC = 512                  # sequence chunk (matmul free dim)
NS = S // SC              # 4 chunks
KT = HID // P             # 32 contraction tiles for the projections
ST = S // P               # 16 sequence tiles of 128
NQK = QH + 2              # col-tiles per core in wqkv: q0..q3, k, v
WCOLS = NQK * P           # 768
GK = 2                    # weight k-tiles per DMA group
NG = KT // GK             # 16 groups
ISQRT_D = float(D) ** -0.5

_CACHE = {}


def _build():
    import concourse.bass as bass
    import concourse.tile as tile
    from concourse import bacc, mybir
    from contextlib import ExitStack

    F32 = mybir.dt.float32
    F16 = mybir.dt.float16
    AF = mybir.ActivationFunctionType

    nc = bacc.Bacc(
        "TRN2",
        target_bir_lowering=False,
        debug=False,
        enable_asserts=False,
        num_devices=NCORES,
    )

    # hT packed: [p, (c*KT + k)*SC + j] = h[c*SC + j, k*P + p]
    hT = nc.dram_tensor("hT", [P, NS * KT * SC], F16, kind="ExternalInput").ap()
    # wqkv packed: [p, k*WCOLS + n] = wq_c[k*P + p, n]
    wqkv = nc.dram_tensor("wqkv", [P, KT * WCOLS], F16, kind="ExternalInput").ap()
    wo = nc.dram_tensor("wo", [QH * D, HID], F16, kind="ExternalInput").ap()
    cos2 = nc.dram_tensor("cos2", [P, S], F16, kind="ExternalInput").ap()
    sinn2 = nc.dram_tensor("sinn2", [P, S], F16, kind="ExternalInput").ap()
    maskd = nc.dram_tensor("maskd", [P, 4 * P], F16, kind="ExternalInput").ap()
    ident = nc.dram_tensor("ident", [P, P], F16, kind="ExternalInput").ap()
    onesd = nc.dram_tensor("onesd", [P, 1], F16, kind="ExternalInput").ap()
    out = nc.dram_tensor("out", [S // NCORES, HID], F16, kind="ExternalOutput").ap()
    # partials: two 256-row halves per chunk (half-sized ReduceScatters keep
    # each SDMA-starvation window short). Separate DRAM tensors so
    # whole-tensor WAR tracking never serializes later o_proj DMA writes
    # behind an in-flight ReduceScatter.
    partH = [
        [nc.dram_tensor(f"part{c}{hh}", [SC // 2, HID], F16).ap() for hh in range(2)]
        for c in range(NS)
    ]
    rsH = [
        [nc.dram_tensor(f"rs{c}{hh}", [32, HID], F16).ap() for hh in range(2)]
        for c in range(NS)
    ]
    # dummy collective: warms ncfw at the real transfer size; gated on the
    # weight-load tail so it moves data only once the DMA rings are quiet.
    dum_in = nc.dram_tensor("dum_in", [SC, HID // 2], F16).ap()
    dum_out = nc.dram_tensor("dum_out", [64, HID // 2], F16).ap()

    groups = [list(range(NCORES))]
    half = P // 2

    with tile.TileContext(nc) as tc:
        with ExitStack() as ctx:
            # ---------------- SBUF pools (whole-kernel lifetime) -------------
            cpool = ctx.enter_context(tc.tile_pool(name="const", bufs=1))
            ppool = ctx.enter_context(tc.tile_pool(name="persist", bufs=1))
            wpool = ctx.enter_context(tc.tile_pool(name="wres", bufs=1))
            h_pool = ctx.enter_context(tc.tile_pool(name="ht", bufs=8))
            rpool = ctx.enter_context(tc.tile_pool(name="ropet", bufs=3))
            ep = ctx.enter_context(tc.tile_pool(name="expp", bufs=6))
            esp = ctx.enter_context(tc.tile_pool(name="esump", bufs=2))
            sp = ctx.enter_context(tc.tile_pool(name="smallp", bufs=2))
            stp = ctx.enter_context(tc.tile_pool(name="stagep", bufs=32))

            mask_t = cpool.tile([P, 4 * P], F16, name="mask_t")
            ident_t = cpool.tile([P, P], F16, name="ident_t")
            ones_t = cpool.tile([P, 1], F16, name="ones_t")
            cos_t = cpool.tile([P, S], F16, name="cos_t")
            sin_t = cpool.tile([P, S], F16, name="sin_t")

            qk = {}
            for n in range(QH + 1):
                for c in range(NS):
                    qk[(n, c)] = ppool.tile(
                        [P, SC], F16, name=f"qk{n}_{c}", tag=f"qk{n}_{c}"
                    )
            vT = [
                ppool.tile([P, SC], F16, name=f"vT{c}", tag=f"vT{c}")
                for c in range(NS)
            ]
            vnat = [
                ppool.tile([P, P], F16, name=f"vn{t}", tag=f"vn{t}")
                for t in range(ST)
            ]
            attnT = {}
            for h in range(QH):
                for c in range(NS):
                    attnT[(h, c)] = ppool.tile(
                        [P, SC], F16, name=f"at{h}_{c}", tag=f"at{h}_{c}"
                    )

            wo_t = [
                wpool.tile([P, HID], F16, name=f"wo{hh}", tag=f"wo{hh}")
                for hh in range(QH)
            ]
            wq_g = {}
            wq_s = {}

            def wslice(k, n):
                # k=0,1 live in single-k tiles (loaded first, so the very
                # first matmul waits on a half-size DMA)
                if k < GK:
                    return wq_s[k][:, n * P : (n + 1) * P]
                g, kk = divmod(k - GK, GK)
                off = kk * WCOLS + n * P
                return wq_g[g][:, off : off + P]

            def emit_rope(c):
                csl = cos_t[:, c * SC : (c + 1) * SC]
                ssl = sin_t[:, c * SC : (c + 1) * SC]
                for n in range(QH + 1):
                    src = qk[(n, c)]
                    swp = rpool.tile([P, SC], F16, name="swp", tag="swp")
                    t1 = rpool.tile([P, SC], F16, name="t1", tag="t1")
                    nc.sync.dma_start(swp[0:half, :], src[half:P, :])
                    nc.sync.dma_start(swp[half:P, :], src[0:half, :])
                    nc.vector.tensor_mul(t1[:], src[:], csl)
                    nc.vector.tensor_mul(swp[:], swp[:], ssl)
                    nc.vector.tensor_add(src[:], t1[:], swp[:])

            # ---------------- stage A: QKV projection + RoPE + vT ------------
            with tc.tile_pool(name="psA", bufs=NQK, space="PSUM") as psA, tc.tile_pool(
                name="pst", bufs=2, space="PSUM"
            ) as pst:
                for c in range(NS):
                    # hT stream for this chunk on the Sync HWDGE queue
                    ht_tiles = []
                    for kk in range(KT // 2):
                        t = h_pool.tile([P, 2 * SC], F16, name="ht_t", tag="ht")
                        off = (c * KT + 2 * kk) * SC
                        nc.sync.dma_start(t[:], hT[:, off : off + 2 * SC])
                        ht_tiles.append(t)
                    if c == 0:
                        # weight stream + consts + wo on the Scalar HWDGE
                        # queue: none of these wait on anything, so ACT rips
                        # through them at t~0 and the rings drain together.
                        for k in range(GK):
                            wq_s[k] = wpool.tile(
                                [P, WCOLS], F16, name=f"wqs{k}", tag=f"wqs{k}"
                            )
                            nc.scalar.dma_start(
                                wq_s[k][:],
                                wqkv[:, k * WCOLS : (k + 1) * WCOLS],
                            )
                        for g in range(NG - 1):
                            wq_g[g] = wpool.tile(
                                [P, GK * WCOLS], F16, name=f"wqg{g}", tag=f"wqg{g}"
                            )
                            base = (GK + g * GK) * WCOLS
                            nc.scalar.dma_start(
                                wq_g[g][:],
                                wqkv[:, base : base + GK * WCOLS],
                            )
                        nc.scalar.dma_start(mask_t[:], maskd)
                        nc.scalar.dma_start(ident_t[:], ident)
                        nc.scalar.dma_start(ones_t[:], onesd)
                        nc.scalar.dma_start(cos_t[:], cos2)
                        nc.scalar.dma_start(sin_t[:], sinn2)
                        for hh in range(QH):
                            nc.scalar.dma_start(
                                wo_t[hh][:], wo[hh * P : (hh + 1) * P, :]
                            )
                        nc.gpsimd.collective_compute(
                            "ReduceScatter",
                            mybir.AluOpType.add,
                            replica_groups=groups,
                            ins=[dum_in[:, :]],
                            outs=[dum_out[:, :]],
                        )
                    ps = [
                        psA.tile([P, SC], F32, name=f"psA{n}", tag="psA")
                        for n in range(NQK)
                    ]
                    for k in range(KT):
                        src = ht_tiles[k // 2][:, (k % 2) * SC : (k % 2 + 1) * SC]
                        for n in range(NQK):
                            nc.tensor.matmul(
                                ps[n][:],
                                wslice(k, n),
                                src,
                                start=(k == 0),
                                stop=(k == KT - 1),
                            )
                    # v first so the transposes can start earliest
                    nc.scalar.copy(vT[c][:], ps[NQK - 1][:])
                    for n in range(QH + 1):
                        nc.scalar.copy(qk[(n, c)][:], ps[n][:])
                    # RoPE for the PREVIOUS chunk: its swap DMAs sit in the
                    # Sync ring after this chunk's hT tiles.
                    if c > 0:
                        emit_rope(c - 1)

                # v transpose to natural [s, d]
                for t in range(ST):
                    c, j = divmod(t, NS)
                    tp = pst.tile([P, P], F16, name="tp", tag="tp")
                    nc.tensor.transpose(
                        tp[:], vT[c][:, j * P : (j + 1) * P], ident_t[:]
                    )
                    nc.vector.tensor_copy(vnat[t][:], tp[:])
                emit_rope(NS - 1)

            # ---------- attention + o_proj + reduce-scatter ----------
            with tc.tile_pool(name="pssc", bufs=3, space="PSUM") as ps_sc, tc.tile_pool(
                name="pssm", bufs=1, space="PSUM"
            ) as ps_sm, tc.tile_pool(
                name="pspv", bufs=2, space="PSUM"
            ) as ps_pv, tc.tile_pool(
                name="psop", bufs=2, space="PSUM"
            ) as ps_op:

                def emit_oproj_group(c, jj, nn):
                    op = ps_op.tile([P, SC], F32, name="op", tag="op")
                    for h in range(QH):
                        nc.tensor.matmul(
                            op[:],
                            attnT[(h, c)][:, jj * P : (jj + 1) * P],
                            wo_t[h][:, nn * SC : (nn + 1) * SC],
                            start=(h == 0),
                            stop=(h == QH - 1),
                        )
                    st = stp.tile([P, SC], F16, name="st", tag="st")
                    nc.vector.tensor_copy(st[:], op[:])
                    dst, row = partH[c][jj // 2], (jj % 2) * P
                    nc.sync.dma_start(
                        dst[row : row + P, nn * SC : (nn + 1) * SC], st[:]
                    )

                def oproj_items(c, jjs):
                    return [
                        (lambda cc=c, j=jj, n=nn: emit_oproj_group(cc, j, n))
                        for jj in jjs
                        for nn in range(HID // SC)
                    ]

                def emit_rs(ins_ap, outs_ap):
                    nc.gpsimd.collective_compute(
                        "ReduceScatter",
                        mybir.AluOpType.add,
                        replica_groups=groups,
                        ins=[ins_ap],
                        outs=[outs_ap],
                    )

                def attention_block(c, q0, W, nsk, diag0, filler, rs_at, end_frac):
                    """Process q-columns [q0:q0+W) of chunk c (all QH heads).

                    filler: list of closures (o_proj groups) paced over the
                    first end_frac of the block's steps. rs_at: {count: fn}
                    fired when that many filler items have been emitted.
                    """
                    total_steps = QH * nsk
                    oi = 0
                    si = 0
                    pending = {}

                    def qk_mm(h, t):
                        kc, kj = divmod(t, NS)
                        ktile = qk[(QH, kc)][:, kj * P : (kj + 1) * P]
                        d = t - diag0
                        lo = d * P if d >= 0 else 0
                        sc_ps = ps_sc.tile([P, SC], F32, name="sc_ps", tag="sc")
                        nc.tensor.matmul(
                            sc_ps[:, lo:W], ktile, qk[(h, c)][:, q0 + lo : q0 + W],
                            start=True, stop=True,
                        )
                        pending[t] = (sc_ps, lo)

                    denom_pending = [None]

                    def emit_denom(h, esum, pv):
                        sm = ps_sm.tile([1, SC], F32, name="sm", tag="sm")
                        nc.tensor.matmul(
                            sm[:, 0:W], ones_t[:], esum[:, 0:W],
                            start=True, stop=True,
                        )
                        smh = sp.tile([1, SC], F32, name="smh", tag="smh")
                        rcp = sp.tile([1, SC], F32, name="rcp", tag="rcp")
                        bc = sp.tile([P, SC], F32, name="bc", tag="bc")
                        nc.scalar.copy(smh[:, 0:W], sm[:, 0:W])
                        nc.vector.reciprocal_approx_fast(rcp[:, 0:W], smh[:, 0:W])
                        nc.gpsimd.partition_broadcast(bc[:, 0:W], rcp[:, 0:W])
                        nc.vector.tensor_mul(
                            attnT[(h, c)][:, q0 : q0 + W], pv[:, 0:W], bc[:, 0:W]
                        )

                    for h in range(QH):
                        esum = esp.tile([P, SC], F16, name="esum", tag="esum")
                        pv = ps_pv.tile([P, SC], F32, name="pv", tag="pv")
                        qk_mm(h, 0)
                        # previous head's softmax denominator: emitted after
                        # this head's first QK so the PE never stalls on the
                        # esum chain at a head boundary.
                        if denom_pending[0] is not None:
                            denom_pending[0]()
                            denom_pending[0] = None
                        for t in range(nsk):
                            if t + 1 < nsk:
                                qk_mm(h, t + 1)
                            sc_ps, lo = pending.pop(t)
                            e = ep.tile([P, SC], F16, name="e", tag="e")
                            nc.scalar.activation(
                                e[:, lo:W], sc_ps[:, lo:W], AF.Exp, scale=ISQRT_D
                            )
                            if t >= diag0:
                                nc.vector.tensor_mul(
                                    e[:, lo : lo + P],
                                    e[:, lo : lo + P],
                                    mask_t[:, 3 * P : 4 * P],
                                )
                            if t == 0:
                                nc.vector.tensor_copy(esum[:, 0:W], e[:, 0:W])
                            else:
                                nc.vector.tensor_add(
                                    esum[:, lo:W], esum[:, lo:W], e[:, lo:W]
                                )
                            nc.tensor.matmul(
                                pv[:, lo:W], vnat[t][:], e[:, lo:W],
                                start=(t == 0), stop=(t == nsk - 1),
                            )
                            si += 1
                            start_si = 6
                            end_si = max(
                                start_si + 1, int(total_steps * end_frac)
                            )
                            frac = (si - start_si) / (end_si - start_si)
                            want = int(len(filler) * min(max(frac, 0.0), 1.0))
                            while oi < want:
                                filler[oi]()
                                oi += 1
                                if oi in rs_at:
                                    rs_at[oi]()
                        denom_pending[0] = (
                            lambda hh=h, es=esum, pp=pv: emit_denom(hh, es, pp)
                        )
                    denom_pending[0]()
                    while oi < len(filler):
                        filler[oi]()
                        oi += 1
                        if oi in rs_at:
                            rs_at[oi]()

                def rs_halves(c):
                    return {
                        16: (lambda cc=c: emit_rs(
                            partH[cc][0][:, :], rsH[cc][0][:, :])),
                        32: (lambda cc=c: emit_rs(
                            partH[cc][1][:, :], rsH[cc][1][:, :])),
                    }

                # chunks 0..2: full 512-wide blocks; chunk c's filler is the
                # previous chunk's o_proj, with a half-RS per 16 groups.
                for c in range(NS - 1):
                    filler = (
                        oproj_items(c - 1, range(QH)) if c > 0 else []
                    )
                    rs_at = rs_halves(c - 1) if c > 0 else {}
                    attention_block(
                        c, 0, SC, QH * c + QH, QH * c, filler, rs_at, 0.65
                    )

                # last chunk: two 256-wide sub-blocks. Sub 0 carries chunk 2's
                # o_proj; sub 1 carries the last chunk's OWN first o_proj half
                # so its RS fires while the PE is still busy.
                attention_block(
                    NS - 1, 0, 256, ST - 2, ST - 4,
                    oproj_items(NS - 2, range(QH)),
                    rs_halves(NS - 2),
                    0.65,
                )
                attention_block(
                    NS - 1, 256, 256, ST, ST - 2,
                    oproj_items(NS - 1, (0, 1)),
                    {16: lambda: emit_rs(
                        partH[NS - 1][0][:, :], rsH[NS - 1][0][:, :])},
                    0.5,
                )
                for fn in oproj_items(NS - 1, (2, 3)):
                    fn()
                emit_rs(partH[NS - 1][1][:, :], rsH[NS - 1][1][:, :])

            # deferred output copies at the tail of the Sync ring
            for c in range(NS):
                for hh in range(2):
                    u = 2 * c + hh
                    nc.sync.dma_start(
                        out[u * 32 : (u + 1) * 32, :], rsH[c][hh][:, :]
                    )

    nc.compile()
    return nc


def _get_nc():
    if "nc" not in _CACHE:
        _CACHE["nc"] = _build()
    return _CACHE["nc"]


def _host_inputs(positions, hidden_states, Wqkv, Wo):
    """Shard + relayout the full inputs for the 8 cores (fp16 device side)."""
    pos = np.asarray(positions).reshape(-1).astype(np.float64)  # [S]
    hs = np.asarray(hidden_states, dtype=np.float32).reshape(S, HID)
    Wqkv = np.asarray(Wqkv, dtype=np.float32)
    Wo = np.asarray(Wo, dtype=np.float32)

    # hT packed: [p, (c*KT + k)*SC + j] = hs[c*SC + j, k*P + p]
    hTp = np.ascontiguousarray(
        hs.reshape(NS, SC, KT, P).transpose(3, 0, 2, 1).reshape(P, NS * KT * SC)
    ).astype(np.float16)

    half = D // 2
    inv_freq = 1.0 / (THETA ** (np.arange(half, dtype=np.float64) / half))
    ang = pos[None, :] * inv_freq[:, None]  # [64, S]
    cos = np.cos(ang)
    sin = np.sin(ang)
    cos2 = np.ascontiguousarray(np.concatenate([cos, cos], axis=0)).astype(
        np.float16
    )
    sinn2 = np.ascontiguousarray(np.concatenate([-sin, sin], axis=0)).astype(
        np.float16
    )

    # causal mask, [sk, sq] orientation: [zeros(128x384) | upper-tri(128x128)].
    maskd = np.concatenate(
        [np.zeros((P, 3 * P), dtype=np.float16),
         np.triu(np.ones((P, P), dtype=np.float16))], axis=1)
    ident = np.eye(P, dtype=np.float16)
    onesd = np.ones((P, 1), dtype=np.float16)

    qb = Wqkv[:, : H * D]
    kb = Wqkv[:, H * D : H * D + KVH * D]
    vb = Wqkv[:, H * D + KVH * D :]

    in_maps = []
    for c in range(NCORES):
        wq_c = np.concatenate(
            [
                qb[:, c * QH * D : (c + 1) * QH * D],
                kb[:, c * D : (c + 1) * D],
                vb[:, c * D : (c + 1) * D],
            ],
            axis=1,
        )
        # packed: [p, k*WCOLS + n] = wq_c[k*P + p, n]
        wq_p = np.ascontiguousarray(
            wq_c.reshape(KT, P, WCOLS).transpose(1, 0, 2).reshape(P, KT * WCOLS)
        ).astype(np.float16)
        wo_c = Wo[c * QH * D : (c + 1) * QH * D, :].astype(np.float16)
        in_maps.append(
            {
                "hT": hTp,
                "wqkv": wq_p,
                "wo": np.ascontiguousarray(wo_c),
                "cos2": cos2,
                "sinn2": sinn2,
                "maskd": maskd,
                "ident": ident,
                "onesd": onesd,
            }
        )
    return in_maps


def _assemble(results):
    full = np.empty((S, HID), dtype=np.float32)
    for r in range(NCORES):
        oc = np.asarray(results[r]["out"], dtype=np.float32)  # [256, HID]
        for c in range(NS):
            for hh in range(2):
                u = 2 * c + hh
                base = SC * c + 256 * hh + 32 * r
                full[base : base + 32, :] = oc[u * 32 : (u + 1) * 32, :]
    return full.reshape(1, S, HID)


def kernel(positions, hidden_states, Wqkv, Wo):
    from concourse.bass_utils import run_bass_kernel_spmd

    nc = _get_nc()
    in_maps = _host_inputs(positions, hidden_states, Wqkv, Wo)
    res = run_bass_kernel_spmd(nc, in_maps, core_ids=list(range(NCORES)))
    return _assemble(res.results)


def kernel_timed(positions, hidden_states, Wqkv, Wo, tmpdir="/tmp/ntff_trace"):
    """Like kernel() but with NTFF profiling; returns (output, exec_time_ns)."""
    import os
    import shutil

    from concourse.bass_utils import run_bass_kernel_spmd

    shutil.rmtree(tmpdir, ignore_errors=True)
    os.makedirs(tmpdir, exist_ok=True)
    nc = _get_nc()
    in_maps = _host_inputs(positions, hidden_states, Wqkv, Wo)
    res = run_bass_kernel_spmd(
        nc, in_maps, core_ids=list(range(NCORES)), trace=True, tmpdir=tmpdir
    )
    return _assemble(res.results), res.exec_time_ns
